# revision 16
# baseline (speedup 1.0000x reference)
"""Trainium2 Bass kernel for Llama SmartKV decode attention (GQA, q_len=1).

Sharding: tensor-parallel over KV heads — core c owns kv head c and its
GQA group of 4 query heads (slices of Wq/Wk/Wv/Wo), plus that head's
quantized KV cache. Each core computes its partial o_proj output; the
host sums the 8 partials (the all-reduce).

Host-side input prep (pure numpy, done once per call):
  - RoPE (cos/sin) and the 1/sqrt(D) score scale are folded into Wq/Wk.
  - k_scale/v_scale are folded into the int8 code cache, stored fp16.
  - All device operands are fp16 (PE runs fp16 at full rate; fp32 PSUM
    accumulation); softmax exp uses a constant bias which cancels in the
    normalization but keeps exp() in fp16 range for any input scale.
"""

import os

os.environ.setdefault("BY_DEFAULT_DISABLE_SUBTILE_DEPS", "1")

import numpy as np

import concourse.bass as bass
import concourse.mybir as mybir
import concourse.tile as tile
from concourse.bass_utils import run_bass_kernel_spmd

H, HKV, D, HID, S = 32, 8, 128, 4096, 32768
G = H // HKV  # 4 query heads per core
NCORES = 8
KC = HID // 128  # 32 contraction chunks for projections
NCH = S // 128  # 256 score/PV chunks of 128 tokens
KTILE = 4096  # tokens per K-cache DMA tile
VCH = 8  # s-chunks per V-cache DMA tile
F16 = mybir.dt.float16
F32 = mybir.dt.float32
EXP_BIAS = -11.0  # exp(s + B): cancels in softmax, keeps fp16 in range

_CACHE = {}


def _reduce_dma_waits(nc):
    """Drop transitively-implied waits from DMA instructions.

    The PSEUDO_DMA_DIRECT2D descriptor holds exactly one wait slot, but
    Tile's sem assignment is not transitively minimal (its optimize_sems
    pass is disabled), so pool-slot-recycling DMAs carry a redundant
    second wait: the WAW wait on the previous slot writer is already
    implied by the engine-reader wait.  We verify implication with a
    vector-clock walk over the scheduled program and delete only waits
    that are provably redundant.
    """
    import bass_rust as _br

    insts = []
    for f in nc.m.functions:
        for bb in f.blocks:
            insts.extend(bb.instructions)

    cum = {}  # sem name -> cumulative value so far in schedule order
    snaps = {}  # sem name -> list of (cumval, knowledge dict)
    streams = {}  # stream key -> knowledge dict (sem name -> value known >=)

    def know_at(sem, val):
        # knowledge of the producer that first brought `sem` to >= val
        for cv, kn in snaps.get(sem, ()):
            if cv >= val:
                return kn
        return None

    for inst in insts:
        si = inst.sync_info
        if si is None:
            continue
        waits = list(si.on_wait)
        ups = list(si.on_update)
        if ups and ups[0].ant_name.startswith(("DMASW", "DMAHW")):
            skey = ups[0].ant_name
        else:
            skey = f"eng:{inst.engine}"
        kn = dict(streams.get(skey, ()))

        imm = [
            w
            for w in waits
            if w.wait_mode == "sem-ge-imm" and w.sync_type == "semaphore"
        ]
        if len(imm) == len(waits) > 1:
            keep = []
            for w in waits:
                others = dict(kn)
                for w2 in waits:
                    if w2 is w:
                        continue
                    others[w2.ant_name] = max(
                        others.get(w2.ant_name, 0), w2.wait_value
                    )
                    k2 = know_at(w2.ant_name, w2.wait_value)
                    if k2:
                        for s, v in k2.items():
                            others[s] = max(others.get(s, 0), v)
                if others.get(w.ant_name, 0) >= w.wait_value:
                    continue  # implied: drop
                keep.append(w)
            if len(keep) < len(waits):
                inst.sync_info = _br.SyncInfo(on_wait=keep, on_update=ups)
                waits = keep

        # fold wait knowledge into this instruction's stream knowledge
        for w in waits:
            if w.wait_mode != "sem-ge-imm" or w.sync_type != "semaphore":
                continue
            kn[w.ant_name] = max(kn.get(w.ant_name, 0), w.wait_value)
            k2 = know_at(w.ant_name, w.wait_value)
            if k2:
                for s, v in k2.items():
                    kn[s] = max(kn.get(s, 0), v)
        for u in ups:
            if u.sync_type != "semaphore":
                continue
            cum[u.ant_name] = cum.get(u.ant_name, 0) + u.update_value
            kn[u.ant_name] = max(kn.get(u.ant_name, 0), cum[u.ant_name])
            snaps.setdefault(u.ant_name, []).append((cum[u.ant_name], kn))
        streams[skey] = kn

    bad = [
        (i.name, type(i).__name__, [(w.ant_name, w.wait_value) for w in i.sync_info.on_wait])
        for i in insts
        if i.sync_info is not None
        and len(i.sync_info.on_wait) > 1
        and type(i).__name__ not in ("InstDrain",)
    ]
    if bad:
        raise RuntimeError(f"instructions still multi-wait: {bad}")


def _build_nc(debug=False):
    nc = bass.Bass()
    hsT = nc.declare_dram_parameter("hsT", [128, KC], F16, isOutput=False)
    wqkv = nc.declare_dram_parameter("wqkv", [KC, 128, 768], F16, isOutput=False)
    kT = nc.declare_dram_parameter("kT", [128, S], F16, isOutput=False)
    v8 = nc.declare_dram_parameter("v8", [S // (128 * VCH), 128, VCH * D], F16, isOutput=False)
    wo = nc.declare_dram_parameter("wo", [G, 128, HID], F16, isOutput=False)
    out = nc.declare_dram_parameter("out", [1, HID], F32, isOutput=True)
    if debug:
        dbg = nc.declare_dram_parameter("dbg", [128, 5280], F32, isOutput=True)

    PS = bass.MemorySpace.PSUM
    with tile.TileContext(nc) as tc:
        with (
            tc.tile_pool(name="const", bufs=1) as cpool,
            tc.tile_pool(name="wqkvp", bufs=3) as wqkv_pool,
            tc.tile_pool(name="kp", bufs=3) as k_pool,
            tc.tile_pool(name="vp", bufs=3) as v_pool,
            tc.tile_pool(name="wop", bufs=1) as wo_pool,
            tc.tile_pool(name="sm", bufs=1) as sm,
        ):
            # ---- load hs ----
            hs_sb = cpool.tile([128, KC], F16)
            nc.gpsimd.dma_start(out=hs_sb, in_=hsT[:, :])
            ebias_sb = cpool.tile([128, 1], F32)
            nc.vector.memset(ebias_sb, EXP_BIAS)
            ones_sb = cpool.tile([128, 1], F32)
            nc.vector.memset(ones_sb, 1.0)
            ones_row = cpool.tile([1, 128], F32)
            nc.vector.memset(ones_row, 1.0)

            qk_sb = sm.tile([128, G + 1], F16)
            vrow_sb = sm.tile([1, D], F16)

            # ---- q/k/v projections (RoPE+scale pre-folded into weights) ----
            # PSUM supports one OPEN accumulation group per bank: each of the
            # 6 concurrently-accumulating outputs (q0..q3, k, v) gets its own
            # bank; the pool scope closes afterwards so later phases reuse them.
            with tc.tile_pool(name="psqk", bufs=1, space=PS) as psqk_pool:
                ps_qk = [
                    psqk_pool.tile([128, 1], F32, name=f"ps_qk{h}", tag=f"qk{h}")
                    for h in range(G + 1)
                ]
                ps_v = psqk_pool.tile([1, D], F32, tag="psv")
                for j in range(KC):
                    w_sb = wqkv_pool.tile([128, 768], F16, tag="wt")
                    nc.gpsimd.dma_start(out=w_sb, in_=wqkv[j])
                    for h in range(G + 1):
                        nc.tensor.matmul(
                            ps_qk[h][:, :],
                            lhsT=w_sb[:, h * 128 : (h + 1) * 128],
                            rhs=hs_sb[:, j : j + 1],
                            start=(j == 0),
                            stop=(j == KC - 1),
                        )
                    nc.tensor.matmul(
                        ps_v[:, :],
                        lhsT=hs_sb[:, j : j + 1],
                        rhs=w_sb[:, 640:768],
                        start=(j == 0),
                        stop=(j == KC - 1),
                    )
                for h in range(G + 1):
                    nc.vector.tensor_copy(out=qk_sb[:, h : h + 1], in_=ps_qk[h])
                nc.vector.tensor_copy(out=vrow_sb, in_=ps_v)

            with (
                tc.tile_pool(name="pssc", bufs=1, space=PS) as pssc_pool,
                tc.tile_pool(name="pspv", bufs=1, space=PS) as pspv_pool,
                tc.tile_pool(name="psms", bufs=1, space=PS) as psms_pool,
            ):
                # ---- current-token score row: s_curT[1, g] = k_cur . q_g ----
                ps_scur = psms_pool.tile([1, G], F32, tag="ms")
                nc.tensor.matmul(
                    ps_scur[:, :], lhsT=qk_sb[:, G : G + 1], rhs=qk_sb[:, 0:G],
                    start=True, stop=True,
                )
                pcur_sb = sm.tile([1, G], F16)
                nc.scalar.activation(
                    out=pcur_sb, in_=ps_scur,
                    func=mybir.ActivationFunctionType.Exp, bias=ebias_sb[:1],
                )
                pcurf_sb = sm.tile([1, G], F32)
                nc.scalar.activation(
                    out=pcurf_sb, in_=ps_scur,
                    func=mybir.ActivationFunctionType.Exp, bias=ebias_sb[:1],
                )

                # ---- scores over the cache: [s, g] layout, 2 PSUM banks ----
                ps_sc = [
                    pssc_pool.tile([128, 512], F32, name=f"ps_sc{b}", tag=f"sc{b}")
                    for b in range(2)
                ]
                probs_sb = [
                    sm.tile([128, 512], F16, name=f"probs{b}", tag=f"pr{b}")
                    for b in range(2)
                ]
                dpart_sb = sm.tile([128, 2 * G], F32)
                kcpt = KTILE // 128  # score chunks per K tile
                for co in range(S // KTILE):
                    k_sb = k_pool.tile([128, KTILE], F16, tag="kt")
                    nc.gpsimd.dma_start(
                        out=k_sb, in_=kT[:, co * KTILE : (co + 1) * KTILE]
                    )
                    for ci in range(kcpt):
                        ch = co * kcpt + ci
                        b, col = ch // 128, (ch % 128) * 4
                        nc.tensor.matmul(
                            ps_sc[b][:, col : col + 4],
                            lhsT=k_sb[:, ci * 128 : (ci + 1) * 128],
                            rhs=qk_sb[:, 0:G],
                            start=True,
                            stop=True,
                        )
                    if (co + 1) * kcpt % 128 == 0:
                        b = ((co + 1) * kcpt - 1) // 128
                        nc.scalar.activation(
                            out=probs_sb[b], in_=ps_sc[b],
                            func=mybir.ActivationFunctionType.Exp, bias=ebias_sb,
                        )
                        # per-(partition, g) partials: reduce over the 128
                        # chunk-columns (stride 4) of the bank
                        nc.vector.reduce_sum(
                            out=dpart_sb[:, b * G : (b + 1) * G],
                            in_=probs_sb[b].rearrange("p (c g) -> p g c", g=G),
                            axis=mybir.AxisListType.X,
                        )

                # ---- PV: outT[d, g] over all 256 chunks + current token ----
                ps_pv = pspv_pool.tile([128, G], F32)
                for co in range(S // (128 * VCH)):
                    v_sb = v_pool.tile([128, VCH * D], F16, tag="vt")
                    nc.gpsimd.dma_start(out=v_sb, in_=v8[co])
                    for ci in range(VCH):
                        ch = co * VCH + ci
                        b, col = ch // 128, (ch % 128) * 4
                        nc.tensor.matmul(
                            ps_pv[:, :],
                            lhsT=v_sb[:, ci * D : (ci + 1) * D],
                            rhs=probs_sb[b][:, col : col + 4],
                            start=(ch == 0),
                            stop=False,
                        )
                nc.tensor.matmul(
                    ps_pv[:, :], lhsT=vrow_sb, rhs=pcur_sb, start=False, stop=True,
                )

                # ---- denominator and normalization ----
                ps_den = psms_pool.tile([1, 2 * G], F32, tag="ms")
                nc.tensor.matmul(
                    ps_den[:, :], lhsT=ones_sb, rhs=dpart_sb, start=True, stop=True,
                )
                den_sb = sm.tile([1, 2 * G], F32)
                nc.vector.tensor_copy(out=den_sb, in_=ps_den)
                dtot_sb = sm.tile([1, G], F32)
                nc.vector.tensor_add(
                    out=dtot_sb, in0=den_sb[:, 0:G], in1=den_sb[:, G : 2 * G]
                )
                nc.vector.tensor_add(out=dtot_sb, in0=dtot_sb, in1=pcurf_sb)
                rden_sb = sm.tile([1, G], F32)
                nc.vector.reciprocal(out=rden_sb, in_=dtot_sb)
                # broadcast rden across partitions on PE (ones outer product)
                # so the normalize's waits all chain through one engine
                ps_bc = psms_pool.tile([128, G], F32, tag="ms")
                nc.tensor.matmul(
                    ps_bc[:, :], lhsT=ones_row, rhs=rden_sb, start=True, stop=True
                )
                bc_sb = sm.tile([128, G], F32)
                nc.vector.tensor_copy(out=bc_sb, in_=ps_bc)
                outn_sb = sm.tile([128, G], F16)
                nc.vector.tensor_mul(out=outn_sb, in0=ps_pv, in1=bc_sb)
                if debug:
                    pv_sb = sm.tile([128, G], F32)
                    nc.vector.tensor_copy(out=pv_sb, in_=ps_pv)

            # ---- o_proj: o[1, HID] = sum_g outT[:, g]^T @ Wo[g] ----
            with tc.tile_pool(name="pso", bufs=2, space=PS) as pso_pool:
                wo_sb = [
                    wo_pool.tile([128, HID], F16, name=f"wo_sb{g}", tag=f"wo{g}")
                    for g in range(G)
                ]
                for g in range(G):
                    nc.gpsimd.dma_start(out=wo_sb[g], in_=wo[g])
                ofin_sb = sm.tile([1, HID], F32)
                for n in range(HID // 512):
                    ps_on = pso_pool.tile([1, 512], F32, tag="on")
                    for g in range(G):
                        nc.tensor.matmul(
                            ps_on[:, :],
                            lhsT=outn_sb[:, g : g + 1],
                            rhs=wo_sb[g][:, n * 512 : (n + 1) * 512],
                            start=(g == 0),
                            stop=(g == G - 1),
                        )
                    nc.scalar.copy(out=ofin_sb[:, n * 512 : (n + 1) * 512], in_=ps_on)
            if not debug:
                nc.gpsimd.dma_start(out=out[:, :], in_=ofin_sb)
            else:
                mega = sm.tile([128, 5280], F32)
                nc.vector.tensor_copy(out=mega[:, 0:512], in_=probs_sb[0])
                nc.vector.tensor_copy(out=mega[:, 512:1024], in_=probs_sb[1])
                nc.vector.tensor_copy(out=mega[:, 1024:1029], in_=qk_sb)
                nc.vector.tensor_copy(out=mega[:, 1029:1037], in_=dpart_sb)
                nc.vector.tensor_copy(out=mega[:, 1037:1041], in_=pv_sb)
                nc.vector.tensor_copy(out=mega[:, 1041:1045], in_=outn_sb)
                nc.vector.tensor_copy(out=mega[0:1, 1045:1173], in_=vrow_sb)
                nc.vector.tensor_copy(out=mega[0:1, 1173:1177], in_=dtot_sb)
                nc.vector.tensor_copy(out=mega[0:1, 1177:1181], in_=pcur_sb)
                nc.vector.tensor_copy(out=mega[0:1, 1184:5280], in_=ofin_sb)
                nc.gpsimd.dma_start(out=dbg[:, :], in_=mega)

    _reduce_dma_waits(nc)
    return nc


def _rope_fold(W, nheads, cos, sin, scale=1.0):
    """Fold RoPE rotation (and an optional scalar) into projection weights."""
    W = W.reshape(HID, nheads, D).astype(np.float32)
    half = D // 2
    Wr = np.empty_like(W)
    Wr[:, :, :half] = cos[:half] * W[:, :, :half] - sin[:half] * W[:, :, half:]
    Wr[:, :, half:] = cos[half:] * W[:, :, half:] + sin[half:] * W[:, :, :half]
    return (Wr * np.float32(scale)).reshape(HID, nheads * D)


def _prep_inputs(hidden_states, k_qx, k_scale, v_qx, v_scale, cos, sin, Wq, Wk, Wv, Wo):
    f16 = np.float16
    hs = np.ascontiguousarray(hidden_states.reshape(HID)).astype(np.float32)
    cos = cos.astype(np.float32)
    sin = sin.astype(np.float32)
    Wq_f = _rope_fold(Wq, H, cos, sin, 1.0 / np.sqrt(D))
    Wk_f = _rope_fold(Wk, HKV, cos, sin)
    hsT = np.ascontiguousarray(hs.reshape(KC, 128).T).astype(f16)

    in_maps = []
    for c in range(NCORES):
        qcols = slice(G * c * D, G * (c + 1) * D)
        kvcols = slice(c * D, (c + 1) * D)
        wqkv = np.concatenate(
            [Wq_f[:, qcols], Wk_f[:, kvcols], Wv[:, kvcols]], axis=1
        ).astype(f16).reshape(KC, 128, 768)
        K = (k_qx[:, c, :].astype(np.float32) * k_scale[:, c, :]).astype(f16)
        kT = np.ascontiguousarray(K.T)
        V = (v_qx[:, c, :].astype(np.float32) * v_scale[:, c, :]).astype(f16)
        v8 = np.ascontiguousarray(
            V.reshape(S // (128 * VCH), VCH, 128, D).transpose(0, 2, 1, 3)
        ).reshape(S // (128 * VCH), 128, VCH * D)
        wo = np.ascontiguousarray(
            Wo[G * c * D : G * (c + 1) * D, :].astype(f16)
        ).reshape(G, 128, HID)
        in_maps.append(
            {"hsT": hsT, "wqkv": wqkv, "kT": kT, "v8": v8, "wo": wo}
        )
    return in_maps


def _run(in_maps, trace=False, **kw):
    if "nc" not in _CACHE:
        _CACHE["nc"] = _build_nc()
    return run_bass_kernel_spmd(
        _CACHE["nc"], in_maps, core_ids=list(range(NCORES)), trace=trace, **kw
    )


def kernel(hidden_states, k_qx, k_scale, v_qx, v_scale, cos, sin, Wq, Wk, Wv, Wo):
    in_maps = _prep_inputs(
        hidden_states, k_qx, k_scale, v_qx, v_scale, cos, sin, Wq, Wk, Wv, Wo
    )
    res = _run(in_maps)
    out = np.zeros((1, 1, HID), np.float32)
    for r in res.results:
        out += r["out"].reshape(1, 1, HID)
    return out


# revision 17
# speedup vs baseline: 1.0782x; 1.0782x over previous
"""Trainium2 Bass kernel for Llama SmartKV decode attention (GQA, q_len=1).

Sharding: tensor-parallel over KV heads — core c owns kv head c and its
GQA group of 4 query heads (slices of Wq/Wk/Wv/Wo), plus that head's
quantized KV cache. Each core computes its partial o_proj output; the
host sums the 8 partials (the all-reduce).

Host-side input prep (pure numpy, done once per call):
  - RoPE (cos/sin) and the 1/sqrt(D) score scale are folded into Wq/Wk.
  - k_scale/v_scale are folded into the int8 code cache, stored fp16.
  - All device operands are fp16 (PE runs fp16 at full rate; fp32 PSUM
    accumulation); softmax exp uses a constant bias which cancels in the
    normalization but keeps exp() in fp16 range for any input scale.
"""

import os

os.environ.setdefault("BY_DEFAULT_DISABLE_SUBTILE_DEPS", "1")

import numpy as np

import concourse.bass as bass
import concourse.mybir as mybir
import concourse.tile as tile
from concourse.bass_utils import run_bass_kernel_spmd

H, HKV, D, HID, S = 32, 8, 128, 4096, 32768
G = H // HKV  # 4 query heads per core
NCORES = 8
KC = HID // 128  # 32 contraction chunks for projections
NCH = S // 128  # 256 score/PV chunks of 128 tokens
KTILE = 4096  # tokens per K-cache DMA tile
VCH = 8  # s-chunks per V-cache DMA tile
F16 = mybir.dt.float16
F32 = mybir.dt.float32
EXP_BIAS = -11.0  # exp(s + B): cancels in softmax, keeps fp16 in range

_CACHE = {}


def _reduce_dma_waits(nc):
    """Drop transitively-implied waits from DMA instructions.

    The PSEUDO_DMA_DIRECT2D descriptor holds exactly one wait slot, but
    Tile's sem assignment is not transitively minimal (its optimize_sems
    pass is disabled), so pool-slot-recycling DMAs carry a redundant
    second wait: the WAW wait on the previous slot writer is already
    implied by the engine-reader wait.  We verify implication with a
    vector-clock walk over the scheduled program and delete only waits
    that are provably redundant.
    """
    import bass_rust as _br

    insts = []
    for f in nc.m.functions:
        for bb in f.blocks:
            insts.extend(bb.instructions)

    cum = {}  # sem name -> cumulative value so far in schedule order
    snaps = {}  # sem name -> list of (cumval, knowledge dict)
    streams = {}  # stream key -> knowledge dict (sem name -> value known >=)

    def know_at(sem, val):
        # knowledge of the producer that first brought `sem` to >= val
        for cv, kn in snaps.get(sem, ()):
            if cv >= val:
                return kn
        return None

    for inst in insts:
        si = inst.sync_info
        if si is None:
            continue
        waits = list(si.on_wait)
        ups = list(si.on_update)
        if ups and ups[0].ant_name.startswith(("DMASW", "DMAHW")):
            skey = ups[0].ant_name
        else:
            skey = f"eng:{inst.engine}"
        kn = dict(streams.get(skey, ()))

        imm = [
            w
            for w in waits
            if w.wait_mode == "sem-ge-imm" and w.sync_type == "semaphore"
        ]
        if len(imm) == len(waits) > 1:
            keep = []
            for w in waits:
                others = dict(kn)
                for w2 in waits:
                    if w2 is w:
                        continue
                    others[w2.ant_name] = max(
                        others.get(w2.ant_name, 0), w2.wait_value
                    )
                    k2 = know_at(w2.ant_name, w2.wait_value)
                    if k2:
                        for s, v in k2.items():
                            others[s] = max(others.get(s, 0), v)
                if others.get(w.ant_name, 0) >= w.wait_value:
                    continue  # implied: drop
                keep.append(w)
            if len(keep) < len(waits):
                inst.sync_info = _br.SyncInfo(on_wait=keep, on_update=ups)
                waits = keep

        # fold wait knowledge into this instruction's stream knowledge
        for w in waits:
            if w.wait_mode != "sem-ge-imm" or w.sync_type != "semaphore":
                continue
            kn[w.ant_name] = max(kn.get(w.ant_name, 0), w.wait_value)
            k2 = know_at(w.ant_name, w.wait_value)
            if k2:
                for s, v in k2.items():
                    kn[s] = max(kn.get(s, 0), v)
        for u in ups:
            if u.sync_type != "semaphore":
                continue
            cum[u.ant_name] = cum.get(u.ant_name, 0) + u.update_value
            kn[u.ant_name] = max(kn.get(u.ant_name, 0), cum[u.ant_name])
            snaps.setdefault(u.ant_name, []).append((cum[u.ant_name], kn))
        streams[skey] = kn

    bad = [
        (i.name, type(i).__name__, [(w.ant_name, w.wait_value) for w in i.sync_info.on_wait])
        for i in insts
        if i.sync_info is not None
        and len(i.sync_info.on_wait) > 1
        and type(i).__name__ not in ("InstDrain",)
    ]
    if bad:
        raise RuntimeError(f"instructions still multi-wait: {bad}")


def _build_nc(debug=False):
    nc = bass.Bass()
    hsT = nc.declare_dram_parameter("hsT", [128, KC], F16, isOutput=False)
    wqkv = nc.declare_dram_parameter("wqkv", [KC, 128, 768], F16, isOutput=False)
    kT = nc.declare_dram_parameter("kT", [128, S], F16, isOutput=False)
    v8 = nc.declare_dram_parameter("v8", [S // (128 * VCH), 128, VCH * D], F16, isOutput=False)
    wo = nc.declare_dram_parameter("wo", [G, 128, HID], F16, isOutput=False)
    out = nc.declare_dram_parameter("out", [1, HID], F32, isOutput=True)
    if debug:
        dbg = nc.declare_dram_parameter("dbg", [128, 5280], F32, isOutput=True)

    PS = bass.MemorySpace.PSUM
    with tile.TileContext(nc) as tc:
        with (
            tc.tile_pool(name="const", bufs=1) as cpool,
            tc.tile_pool(name="wqkvp", bufs=3) as wqkv_pool,
            tc.tile_pool(name="kp", bufs=3) as k_pool,
            tc.tile_pool(name="vp", bufs=3) as v_pool,
            tc.tile_pool(name="wop", bufs=1) as wo_pool,
            tc.tile_pool(name="sm", bufs=1) as sm,
        ):
            # ---- load hs ----
            hs_sb = cpool.tile([128, KC], F16)
            nc.sync.dma_start(out=hs_sb, in_=hsT[:, :])
            ebias_sb = cpool.tile([128, 1], F32)
            nc.vector.memset(ebias_sb, EXP_BIAS)
            ones_sb = cpool.tile([128, 1], F32)
            nc.vector.memset(ones_sb, 1.0)
            ones_row = cpool.tile([1, 128], F32)
            nc.vector.memset(ones_row, 1.0)

            qk_sb = sm.tile([128, G + 1], F16)
            vrow_sb = sm.tile([1, D], F16)

            # ---- q/k/v projections (RoPE+scale pre-folded into weights) ----
            # PSUM supports one OPEN accumulation group per bank: each of the
            # 6 concurrently-accumulating outputs (q0..q3, k, v) gets its own
            # bank; the pool scope closes afterwards so later phases reuse them.
            with tc.tile_pool(name="psqk", bufs=1, space=PS) as psqk_pool:
                ps_qk = [
                    psqk_pool.tile([128, 1], F32, name=f"ps_qk{h}", tag=f"qk{h}")
                    for h in range(G + 1)
                ]
                ps_v = psqk_pool.tile([1, D], F32, tag="psv")
                for j in range(KC):
                    w_sb = wqkv_pool.tile([128, 768], F16, tag="wt")
                    nc.sync.dma_start(out=w_sb, in_=wqkv[j])
                    for h in range(G + 1):
                        nc.tensor.matmul(
                            ps_qk[h][:, :],
                            lhsT=w_sb[:, h * 128 : (h + 1) * 128],
                            rhs=hs_sb[:, j : j + 1],
                            start=(j == 0),
                            stop=(j == KC - 1),
                        )
                    nc.tensor.matmul(
                        ps_v[:, :],
                        lhsT=hs_sb[:, j : j + 1],
                        rhs=w_sb[:, 640:768],
                        start=(j == 0),
                        stop=(j == KC - 1),
                    )
                for h in range(G + 1):
                    nc.vector.tensor_copy(out=qk_sb[:, h : h + 1], in_=ps_qk[h])
                nc.vector.tensor_copy(out=vrow_sb, in_=ps_v)

            # prefetch o_proj weights early so the tail isn't DMA-bound
            wo_sb = [
                wo_pool.tile([128, HID], F16, name=f"wo_sb{g}", tag=f"wo{g}")
                for g in range(G)
            ]
            for g in range(G):
                nc.sync.dma_start(out=wo_sb[g], in_=wo[g])

            with (
                tc.tile_pool(name="pssc", bufs=1, space=PS) as pssc_pool,
                tc.tile_pool(name="pspv", bufs=1, space=PS) as pspv_pool,
                tc.tile_pool(name="psms", bufs=1, space=PS) as psms_pool,
            ):
                # ---- current-token score row: s_curT[1, g] = k_cur . q_g ----
                ps_scur = psms_pool.tile([1, G], F32, tag="ms")
                nc.tensor.matmul(
                    ps_scur[:, :], lhsT=qk_sb[:, G : G + 1], rhs=qk_sb[:, 0:G],
                    start=True, stop=True,
                )
                pcur_sb = sm.tile([1, G], F16)
                nc.scalar.activation(
                    out=pcur_sb, in_=ps_scur,
                    func=mybir.ActivationFunctionType.Exp, bias=ebias_sb[:1],
                )
                pcurf_sb = sm.tile([1, G], F32)
                nc.scalar.activation(
                    out=pcurf_sb, in_=ps_scur,
                    func=mybir.ActivationFunctionType.Exp, bias=ebias_sb[:1],
                )

                # ---- scores over the cache: [s, g] layout, 2 PSUM banks ----
                ps_sc = [
                    pssc_pool.tile([128, 512], F32, name=f"ps_sc{b}", tag=f"sc{b}")
                    for b in range(2)
                ]
                probs_sb = [
                    sm.tile([128, 512], F16, name=f"probs{b}", tag=f"pr{b}")
                    for b in range(2)
                ]
                dpart_sb = sm.tile([128, 2 * G], F32)
                kcpt = KTILE // 128  # score chunks per K tile
                for co in range(S // KTILE):
                    k_sb = k_pool.tile([128, KTILE], F16, tag="kt")
                    nc.sync.dma_start(
                        out=k_sb, in_=kT[:, co * KTILE : (co + 1) * KTILE]
                    )
                    for ci in range(kcpt):
                        ch = co * kcpt + ci
                        b, col = ch // 128, (ch % 128) * 4
                        nc.tensor.matmul(
                            ps_sc[b][:, col : col + 4],
                            lhsT=k_sb[:, ci * 128 : (ci + 1) * 128],
                            rhs=qk_sb[:, 0:G],
                            start=True,
                            stop=True,
                        )
                    if (co + 1) * kcpt % 128 == 0:
                        b = ((co + 1) * kcpt - 1) // 128
                        nc.scalar.activation(
                            out=probs_sb[b], in_=ps_sc[b],
                            func=mybir.ActivationFunctionType.Exp, bias=ebias_sb,
                        )
                        # per-(partition, g) partials: reduce over the 128
                        # chunk-columns (stride 4) of the bank
                        nc.vector.reduce_sum(
                            out=dpart_sb[:, b * G : (b + 1) * G],
                            in_=probs_sb[b].rearrange("p (c g) -> p g c", g=G),
                            axis=mybir.AxisListType.X,
                        )

                # ---- PV: outT[d, g] over all 256 chunks + current token ----
                ps_pv = pspv_pool.tile([128, G], F32)
                for co in range(S // (128 * VCH)):
                    v_sb = v_pool.tile([128, VCH * D], F16, tag="vt")
                    nc.sync.dma_start(out=v_sb, in_=v8[co])
                    for ci in range(VCH):
                        ch = co * VCH + ci
                        b, col = ch // 128, (ch % 128) * 4
                        nc.tensor.matmul(
                            ps_pv[:, :],
                            lhsT=v_sb[:, ci * D : (ci + 1) * D],
                            rhs=probs_sb[b][:, col : col + 4],
                            start=(ch == 0),
                            stop=False,
                        )
                nc.tensor.matmul(
                    ps_pv[:, :], lhsT=vrow_sb, rhs=pcur_sb, start=False, stop=True,
                )

                # ---- denominator and normalization ----
                ps_den = psms_pool.tile([1, 2 * G], F32, tag="ms")
                nc.tensor.matmul(
                    ps_den[:, :], lhsT=ones_sb, rhs=dpart_sb, start=True, stop=True,
                )
                den_sb = sm.tile([1, 2 * G], F32)
                nc.vector.tensor_copy(out=den_sb, in_=ps_den)
                dtot_sb = sm.tile([1, G], F32)
                nc.vector.tensor_add(
                    out=dtot_sb, in0=den_sb[:, 0:G], in1=den_sb[:, G : 2 * G]
                )
                nc.vector.tensor_add(out=dtot_sb, in0=dtot_sb, in1=pcurf_sb)
                rden_sb = sm.tile([1, G], F32)
                nc.vector.reciprocal(out=rden_sb, in_=dtot_sb)
                # broadcast rden across partitions on PE (ones outer product)
                # so the normalize's waits all chain through one engine
                ps_bc = psms_pool.tile([128, G], F32, tag="ms")
                nc.tensor.matmul(
                    ps_bc[:, :], lhsT=ones_row, rhs=rden_sb, start=True, stop=True
                )
                bc_sb = sm.tile([128, G], F32)
                nc.vector.tensor_copy(out=bc_sb, in_=ps_bc)
                outn_sb = sm.tile([128, G], F16)
                nc.vector.tensor_mul(out=outn_sb, in0=ps_pv, in1=bc_sb)
                if debug:
                    pv_sb = sm.tile([128, G], F32)
                    nc.vector.tensor_copy(out=pv_sb, in_=ps_pv)

            # ---- o_proj: o[1, HID] = sum_g outT[:, g]^T @ Wo[g] ----
            with tc.tile_pool(name="pso", bufs=2, space=PS) as pso_pool:
                ofin_sb = sm.tile([1, HID], F32)
                for n in range(HID // 512):
                    ps_on = pso_pool.tile([1, 512], F32, tag="on")
                    for g in range(G):
                        nc.tensor.matmul(
                            ps_on[:, :],
                            lhsT=outn_sb[:, g : g + 1],
                            rhs=wo_sb[g][:, n * 512 : (n + 1) * 512],
                            start=(g == 0),
                            stop=(g == G - 1),
                        )
                    nc.scalar.copy(out=ofin_sb[:, n * 512 : (n + 1) * 512], in_=ps_on)
            if not debug:
                nc.gpsimd.dma_start(out=out[:, :], in_=ofin_sb)
            else:
                mega = sm.tile([128, 5280], F32)
                nc.vector.tensor_copy(out=mega[:, 0:512], in_=probs_sb[0])
                nc.vector.tensor_copy(out=mega[:, 512:1024], in_=probs_sb[1])
                nc.vector.tensor_copy(out=mega[:, 1024:1029], in_=qk_sb)
                nc.vector.tensor_copy(out=mega[:, 1029:1037], in_=dpart_sb)
                nc.vector.tensor_copy(out=mega[:, 1037:1041], in_=pv_sb)
                nc.vector.tensor_copy(out=mega[:, 1041:1045], in_=outn_sb)
                nc.vector.tensor_copy(out=mega[0:1, 1045:1173], in_=vrow_sb)
                nc.vector.tensor_copy(out=mega[0:1, 1173:1177], in_=dtot_sb)
                nc.vector.tensor_copy(out=mega[0:1, 1177:1181], in_=pcur_sb)
                nc.vector.tensor_copy(out=mega[0:1, 1184:5280], in_=ofin_sb)
                nc.gpsimd.dma_start(out=dbg[:, :], in_=mega)

    _reduce_dma_waits(nc)
    return nc


def _rope_fold(W, nheads, cos, sin, scale=1.0):
    """Fold RoPE rotation (and an optional scalar) into projection weights."""
    W = W.reshape(HID, nheads, D).astype(np.float32)
    half = D // 2
    Wr = np.empty_like(W)
    Wr[:, :, :half] = cos[:half] * W[:, :, :half] - sin[:half] * W[:, :, half:]
    Wr[:, :, half:] = cos[half:] * W[:, :, half:] + sin[half:] * W[:, :, :half]
    return (Wr * np.float32(scale)).reshape(HID, nheads * D)


def _prep_inputs(hidden_states, k_qx, k_scale, v_qx, v_scale, cos, sin, Wq, Wk, Wv, Wo):
    f16 = np.float16
    hs = np.ascontiguousarray(hidden_states.reshape(HID)).astype(np.float32)
    cos = cos.astype(np.float32)
    sin = sin.astype(np.float32)
    Wq_f = _rope_fold(Wq, H, cos, sin, 1.0 / np.sqrt(D))
    Wk_f = _rope_fold(Wk, HKV, cos, sin)
    hsT = np.ascontiguousarray(hs.reshape(KC, 128).T).astype(f16)

    in_maps = []
    for c in range(NCORES):
        qcols = slice(G * c * D, G * (c + 1) * D)
        kvcols = slice(c * D, (c + 1) * D)
        wqkv = np.concatenate(
            [Wq_f[:, qcols], Wk_f[:, kvcols], Wv[:, kvcols]], axis=1
        ).astype(f16).reshape(KC, 128, 768)
        K = (k_qx[:, c, :].astype(np.float32) * k_scale[:, c, :]).astype(f16)
        kT = np.ascontiguousarray(K.T)
        V = (v_qx[:, c, :].astype(np.float32) * v_scale[:, c, :]).astype(f16)
        v8 = np.ascontiguousarray(
            V.reshape(S // (128 * VCH), VCH, 128, D).transpose(0, 2, 1, 3)
        ).reshape(S // (128 * VCH), 128, VCH * D)
        wo = np.ascontiguousarray(
            Wo[G * c * D : G * (c + 1) * D, :].astype(f16)
        ).reshape(G, 128, HID)
        in_maps.append(
            {"hsT": hsT, "wqkv": wqkv, "kT": kT, "v8": v8, "wo": wo}
        )
    return in_maps


def _run(in_maps, trace=False, **kw):
    if "nc" not in _CACHE:
        _CACHE["nc"] = _build_nc()
    return run_bass_kernel_spmd(
        _CACHE["nc"], in_maps, core_ids=list(range(NCORES)), trace=trace, **kw
    )


def kernel(hidden_states, k_qx, k_scale, v_qx, v_scale, cos, sin, Wq, Wk, Wv, Wo):
    in_maps = _prep_inputs(
        hidden_states, k_qx, k_scale, v_qx, v_scale, cos, sin, Wq, Wk, Wv, Wo
    )
    res = _run(in_maps)
    out = np.zeros((1, 1, HID), np.float32)
    for r in res.results:
        out += r["out"].reshape(1, 1, HID)
    return out


# revision 18
# speedup vs baseline: 1.2939x; 1.2001x over previous
"""Trainium2 Bass kernel for Llama SmartKV decode attention (GQA, q_len=1).

Sharding: tensor-parallel over KV heads — core c owns kv head c and its
GQA group of 4 query heads (slices of Wq/Wk/Wv/Wo), plus that head's
quantized KV cache. Each core computes its partial o_proj output; the
host sums the 8 partials (the all-reduce).

Host-side input prep (pure numpy, done once per call):
  - RoPE (cos/sin) and the 1/sqrt(D) score scale are folded into Wq/Wk.
  - k_scale/v_scale are folded into the int8 code cache, stored fp16.
  - All device operands are fp16 (PE runs fp16 at full rate; fp32 PSUM
    accumulation); softmax exp uses a constant bias which cancels in the
    normalization but keeps exp() in fp16 range for any input scale.
"""

import os

os.environ.setdefault("BY_DEFAULT_DISABLE_SUBTILE_DEPS", "1")

import numpy as np

import concourse.bass as bass
import concourse.mybir as mybir
import concourse.tile as tile
from concourse.bass_utils import run_bass_kernel_spmd

H, HKV, D, HID, S = 32, 8, 128, 4096, 32768
G = H // HKV  # 4 query heads per core
NCORES = 8
KC = HID // 128  # 32 contraction chunks for projections
NCH = S // 128  # 256 score/PV chunks of 128 tokens
KTILE = 8192  # tokens per K-cache DMA tile
VCH = 32  # s-chunks per V-cache DMA tile
WJ = 4  # projection j-chunks per wqkv DMA tile
F16 = mybir.dt.float16
F32 = mybir.dt.float32
EXP_BIAS = -11.0  # exp(s + B): cancels in softmax, keeps fp16 in range

_CACHE = {}


def _reduce_dma_waits(nc):
    """Drop transitively-implied waits from DMA instructions.

    The PSEUDO_DMA_DIRECT2D descriptor holds exactly one wait slot, but
    Tile's sem assignment is not transitively minimal (its optimize_sems
    pass is disabled), so pool-slot-recycling DMAs carry a redundant
    second wait: the WAW wait on the previous slot writer is already
    implied by the engine-reader wait.  We verify implication with a
    vector-clock walk over the scheduled program and delete only waits
    that are provably redundant.
    """
    import bass_rust as _br

    insts = []
    for f in nc.m.functions:
        for bb in f.blocks:
            insts.extend(bb.instructions)

    cum = {}  # sem name -> cumulative value so far in schedule order
    snaps = {}  # sem name -> list of (cumval, knowledge dict)
    streams = {}  # stream key -> knowledge dict (sem name -> value known >=)

    def know_at(sem, val):
        # knowledge of the producer that first brought `sem` to >= val
        for cv, kn in snaps.get(sem, ()):
            if cv >= val:
                return kn
        return None

    for inst in insts:
        si = inst.sync_info
        if si is None:
            continue
        waits = list(si.on_wait)
        ups = list(si.on_update)
        if ups and ups[0].ant_name.startswith(("DMASW", "DMAHW")):
            skey = ups[0].ant_name
        else:
            skey = f"eng:{inst.engine}"
        kn = dict(streams.get(skey, ()))

        imm = [
            w
            for w in waits
            if w.wait_mode == "sem-ge-imm" and w.sync_type == "semaphore"
        ]
        if len(imm) == len(waits) > 1:
            keep = []
            for w in waits:
                others = dict(kn)
                for w2 in waits:
                    if w2 is w:
                        continue
                    others[w2.ant_name] = max(
                        others.get(w2.ant_name, 0), w2.wait_value
                    )
                    k2 = know_at(w2.ant_name, w2.wait_value)
                    if k2:
                        for s, v in k2.items():
                            others[s] = max(others.get(s, 0), v)
                if others.get(w.ant_name, 0) >= w.wait_value:
                    continue  # implied: drop
                keep.append(w)
            if len(keep) < len(waits):
                inst.sync_info = _br.SyncInfo(on_wait=keep, on_update=ups)
                waits = keep

        # fold wait knowledge into this instruction's stream knowledge
        for w in waits:
            if w.wait_mode != "sem-ge-imm" or w.sync_type != "semaphore":
                continue
            kn[w.ant_name] = max(kn.get(w.ant_name, 0), w.wait_value)
            k2 = know_at(w.ant_name, w.wait_value)
            if k2:
                for s, v in k2.items():
                    kn[s] = max(kn.get(s, 0), v)
        for u in ups:
            if u.sync_type != "semaphore":
                continue
            cum[u.ant_name] = cum.get(u.ant_name, 0) + u.update_value
            kn[u.ant_name] = max(kn.get(u.ant_name, 0), cum[u.ant_name])
            snaps.setdefault(u.ant_name, []).append((cum[u.ant_name], kn))
        streams[skey] = kn

    bad = [
        (i.name, type(i).__name__, [(w.ant_name, w.wait_value) for w in i.sync_info.on_wait])
        for i in insts
        if i.sync_info is not None
        and len(i.sync_info.on_wait) > 1
        and type(i).__name__ not in ("InstDrain",)
    ]
    if bad:
        raise RuntimeError(f"instructions still multi-wait: {bad}")


def _build_nc(debug=False):
    nc = bass.Bass()
    hsT = nc.declare_dram_parameter("hsT", [128, KC], F16, isOutput=False)
    wqkv = nc.declare_dram_parameter("wqkv", [KC // WJ, 128, WJ * 768], F16, isOutput=False)
    kT = nc.declare_dram_parameter("kT", [128, S], F16, isOutput=False)
    v8 = nc.declare_dram_parameter("v8", [S // (128 * VCH), 128, VCH * D], F16, isOutput=False)
    wo = nc.declare_dram_parameter("wo", [G, 128, HID], F16, isOutput=False)
    out = nc.declare_dram_parameter("out", [1, HID], F32, isOutput=True)
    if debug:
        dbg = nc.declare_dram_parameter("dbg", [128, 5280], F32, isOutput=True)

    PS = bass.MemorySpace.PSUM
    with tile.TileContext(nc) as tc:
        with (
            tc.tile_pool(name="const", bufs=1) as cpool,
            tc.tile_pool(name="wqkvp", bufs=3) as wqkv_pool,
            tc.tile_pool(name="kp", bufs=3) as k_pool,
            tc.tile_pool(name="vp", bufs=3) as v_pool,
            tc.tile_pool(name="wop", bufs=1) as wo_pool,
            tc.tile_pool(name="sm", bufs=1) as sm,
        ):
            # ---- load hs ----
            hs_sb = cpool.tile([128, KC], F16)
            nc.sync.dma_start(out=hs_sb, in_=hsT[:, :])
            ebias_sb = cpool.tile([128, 1], F32)
            nc.vector.memset(ebias_sb, EXP_BIAS)
            ones_sb = cpool.tile([128, 1], F32)
            nc.vector.memset(ones_sb, 1.0)
            ones_row = cpool.tile([1, 128], F32)
            nc.vector.memset(ones_row, 1.0)

            qk_sb = sm.tile([128, G + 1], F16)
            vrow_sb = sm.tile([1, D], F16)

            # ---- q/k/v projections (RoPE+scale pre-folded into weights) ----
            # PSUM supports one OPEN accumulation group per bank: each of the
            # 6 concurrently-accumulating outputs (q0..q3, k, v) gets its own
            # bank; the pool scope closes afterwards so later phases reuse them.
            with tc.tile_pool(name="psqk", bufs=1, space=PS) as psqk_pool:
                ps_qk = [
                    psqk_pool.tile([128, 1], F32, name=f"ps_qk{h}", tag=f"qk{h}")
                    for h in range(G + 1)
                ]
                ps_v = psqk_pool.tile([1, D], F32, tag="psv")
                for jj in range(KC // WJ):
                    w_sb = wqkv_pool.tile([128, WJ * 768], F16, tag="wt")
                    nc.gpsimd.dma_start(out=w_sb, in_=wqkv[jj])
                    for c in range(WJ):
                        j = jj * WJ + c
                        for h in range(G + 1):
                            nc.tensor.matmul(
                                ps_qk[h][:, :],
                                lhsT=w_sb[:, c * 768 + h * 128 : c * 768 + (h + 1) * 128],
                                rhs=hs_sb[:, j : j + 1],
                                start=(j == 0),
                                stop=(j == KC - 1),
                            )
                        nc.tensor.matmul(
                            ps_v[:, :],
                            lhsT=hs_sb[:, j : j + 1],
                            rhs=w_sb[:, c * 768 + 640 : c * 768 + 768],
                            start=(j == 0),
                            stop=(j == KC - 1),
                        )
                for h in range(G + 1):
                    nc.vector.tensor_copy(out=qk_sb[:, h : h + 1], in_=ps_qk[h])
                nc.vector.tensor_copy(out=vrow_sb, in_=ps_v)

            # prefetch o_proj weights early so the tail isn't DMA-bound
            wo_sb = [
                wo_pool.tile([128, HID], F16, name=f"wo_sb{g}", tag=f"wo{g}")
                for g in range(G)
            ]
            for g in range(G):
                nc.gpsimd.dma_start(out=wo_sb[g], in_=wo[g])

            with (
                tc.tile_pool(name="pssc", bufs=1, space=PS) as pssc_pool,
                tc.tile_pool(name="pspv", bufs=1, space=PS) as pspv_pool,
                tc.tile_pool(name="psms", bufs=1, space=PS) as psms_pool,
            ):
                # ---- current-token score row: s_curT[1, g] = k_cur . q_g ----
                ps_scur = psms_pool.tile([1, G], F32, tag="ms")
                nc.tensor.matmul(
                    ps_scur[:, :], lhsT=qk_sb[:, G : G + 1], rhs=qk_sb[:, 0:G],
                    start=True, stop=True,
                )
                pcur_sb = sm.tile([1, G], F16)
                nc.scalar.activation(
                    out=pcur_sb, in_=ps_scur,
                    func=mybir.ActivationFunctionType.Exp, bias=ebias_sb[:1],
                )
                pcurf_sb = sm.tile([1, G], F32)
                nc.scalar.activation(
                    out=pcurf_sb, in_=ps_scur,
                    func=mybir.ActivationFunctionType.Exp, bias=ebias_sb[:1],
                )

                # ---- scores over the cache: [s, g] layout, 2 PSUM banks ----
                ps_sc = [
                    pssc_pool.tile([128, 512], F32, name=f"ps_sc{b}", tag=f"sc{b}")
                    for b in range(2)
                ]
                probs_sb = [
                    sm.tile([128, 512], F16, name=f"probs{b}", tag=f"pr{b}")
                    for b in range(2)
                ]
                dpart_sb = sm.tile([128, 2 * G], F32)
                kcpt = KTILE // 128  # score chunks per K tile
                for co in range(S // KTILE):
                    k_sb = k_pool.tile([128, KTILE], F16, tag="kt")
                    nc.sync.dma_start(
                        out=k_sb, in_=kT[:, co * KTILE : (co + 1) * KTILE]
                    )
                    for ci in range(kcpt):
                        ch = co * kcpt + ci
                        b, col = ch // 128, (ch % 128) * 4
                        nc.tensor.matmul(
                            ps_sc[b][:, col : col + 4],
                            lhsT=k_sb[:, ci * 128 : (ci + 1) * 128],
                            rhs=qk_sb[:, 0:G],
                            start=True,
                            stop=True,
                        )
                    if (co + 1) * kcpt % 128 == 0:
                        b = ((co + 1) * kcpt - 1) // 128
                        nc.scalar.activation(
                            out=probs_sb[b], in_=ps_sc[b],
                            func=mybir.ActivationFunctionType.Exp, bias=ebias_sb,
                        )
                        # per-(partition, g) partials: reduce over the 128
                        # chunk-columns (stride 4) of the bank
                        nc.vector.reduce_sum(
                            out=dpart_sb[:, b * G : (b + 1) * G],
                            in_=probs_sb[b].rearrange("p (c g) -> p g c", g=G),
                            axis=mybir.AxisListType.X,
                        )

                # ---- PV: outT[d, g] over all 256 chunks + current token ----
                ps_pv = pspv_pool.tile([128, G], F32)
                for co in range(S // (128 * VCH)):
                    v_sb = v_pool.tile([128, VCH * D], F16, tag="vt")
                    nc.scalar.dma_start(out=v_sb, in_=v8[co])
                    for ci in range(VCH):
                        ch = co * VCH + ci
                        b, col = ch // 128, (ch % 128) * 4
                        nc.tensor.matmul(
                            ps_pv[:, :],
                            lhsT=v_sb[:, ci * D : (ci + 1) * D],
                            rhs=probs_sb[b][:, col : col + 4],
                            start=(ch == 0),
                            stop=False,
                        )
                nc.tensor.matmul(
                    ps_pv[:, :], lhsT=vrow_sb, rhs=pcur_sb, start=False, stop=True,
                )

                # ---- denominator and normalization ----
                ps_den = psms_pool.tile([1, 2 * G], F32, tag="ms")
                nc.tensor.matmul(
                    ps_den[:, :], lhsT=ones_sb, rhs=dpart_sb, start=True, stop=True,
                )
                den_sb = sm.tile([1, 2 * G], F32)
                nc.vector.tensor_copy(out=den_sb, in_=ps_den)
                dtot_sb = sm.tile([1, G], F32)
                nc.vector.tensor_add(
                    out=dtot_sb, in0=den_sb[:, 0:G], in1=den_sb[:, G : 2 * G]
                )
                nc.vector.tensor_add(out=dtot_sb, in0=dtot_sb, in1=pcurf_sb)
                rden_sb = sm.tile([1, G], F32)
                nc.vector.reciprocal(out=rden_sb, in_=dtot_sb)
                # broadcast rden across partitions on PE (ones outer product)
                # so the normalize's waits all chain through one engine
                ps_bc = psms_pool.tile([128, G], F32, tag="ms")
                nc.tensor.matmul(
                    ps_bc[:, :], lhsT=ones_row, rhs=rden_sb, start=True, stop=True
                )
                bc_sb = sm.tile([128, G], F32)
                nc.vector.tensor_copy(out=bc_sb, in_=ps_bc)
                outn_sb = sm.tile([128, G], F16)
                nc.vector.tensor_mul(out=outn_sb, in0=ps_pv, in1=bc_sb)
                if debug:
                    pv_sb = sm.tile([128, G], F32)
                    nc.vector.tensor_copy(out=pv_sb, in_=ps_pv)

            # ---- o_proj: o[1, HID] = sum_g outT[:, g]^T @ Wo[g] ----
            with tc.tile_pool(name="pso", bufs=2, space=PS) as pso_pool:
                ofin_sb = sm.tile([1, HID], F32)
                for n in range(HID // 512):
                    ps_on = pso_pool.tile([1, 512], F32, tag="on")
                    for g in range(G):
                        nc.tensor.matmul(
                            ps_on[:, :],
                            lhsT=outn_sb[:, g : g + 1],
                            rhs=wo_sb[g][:, n * 512 : (n + 1) * 512],
                            start=(g == 0),
                            stop=(g == G - 1),
                        )
                    nc.scalar.copy(out=ofin_sb[:, n * 512 : (n + 1) * 512], in_=ps_on)
            if not debug:
                nc.gpsimd.dma_start(out=out[:, :], in_=ofin_sb)
            else:
                mega = sm.tile([128, 5280], F32)
                nc.vector.tensor_copy(out=mega[:, 0:512], in_=probs_sb[0])
                nc.vector.tensor_copy(out=mega[:, 512:1024], in_=probs_sb[1])
                nc.vector.tensor_copy(out=mega[:, 1024:1029], in_=qk_sb)
                nc.vector.tensor_copy(out=mega[:, 1029:1037], in_=dpart_sb)
                nc.vector.tensor_copy(out=mega[:, 1037:1041], in_=pv_sb)
                nc.vector.tensor_copy(out=mega[:, 1041:1045], in_=outn_sb)
                nc.vector.tensor_copy(out=mega[0:1, 1045:1173], in_=vrow_sb)
                nc.vector.tensor_copy(out=mega[0:1, 1173:1177], in_=dtot_sb)
                nc.vector.tensor_copy(out=mega[0:1, 1177:1181], in_=pcur_sb)
                nc.vector.tensor_copy(out=mega[0:1, 1184:5280], in_=ofin_sb)
                nc.gpsimd.dma_start(out=dbg[:, :], in_=mega)

    _reduce_dma_waits(nc)
    return nc


def _rope_fold(W, nheads, cos, sin, scale=1.0):
    """Fold RoPE rotation (and an optional scalar) into projection weights."""
    W = W.reshape(HID, nheads, D).astype(np.float32)
    half = D // 2
    Wr = np.empty_like(W)
    Wr[:, :, :half] = cos[:half] * W[:, :, :half] - sin[:half] * W[:, :, half:]
    Wr[:, :, half:] = cos[half:] * W[:, :, half:] + sin[half:] * W[:, :, :half]
    return (Wr * np.float32(scale)).reshape(HID, nheads * D)


def _prep_inputs(hidden_states, k_qx, k_scale, v_qx, v_scale, cos, sin, Wq, Wk, Wv, Wo):
    f16 = np.float16
    hs = np.ascontiguousarray(hidden_states.reshape(HID)).astype(np.float32)
    cos = cos.astype(np.float32)
    sin = sin.astype(np.float32)
    Wq_f = _rope_fold(Wq, H, cos, sin, 1.0 / np.sqrt(D))
    Wk_f = _rope_fold(Wk, HKV, cos, sin)
    hsT = np.ascontiguousarray(hs.reshape(KC, 128).T).astype(f16)

    in_maps = []
    for c in range(NCORES):
        qcols = slice(G * c * D, G * (c + 1) * D)
        kvcols = slice(c * D, (c + 1) * D)
        wqkv = np.ascontiguousarray(
            np.concatenate([Wq_f[:, qcols], Wk_f[:, kvcols], Wv[:, kvcols]], axis=1)
            .astype(f16)
            .reshape(KC // 4, 4, 128, 768)
            .transpose(0, 2, 1, 3)
        ).reshape(KC // 4, 128, 4 * 768)
        K = (k_qx[:, c, :].astype(np.float32) * k_scale[:, c, :]).astype(f16)
        kT = np.ascontiguousarray(K.T)
        V = (v_qx[:, c, :].astype(np.float32) * v_scale[:, c, :]).astype(f16)
        v8 = np.ascontiguousarray(
            V.reshape(S // (128 * VCH), VCH, 128, D).transpose(0, 2, 1, 3)
        ).reshape(S // (128 * VCH), 128, VCH * D)
        wo = np.ascontiguousarray(
            Wo[G * c * D : G * (c + 1) * D, :].astype(f16)
        ).reshape(G, 128, HID)
        in_maps.append(
            {"hsT": hsT, "wqkv": wqkv, "kT": kT, "v8": v8, "wo": wo}
        )
    return in_maps


def _run(in_maps, trace=False, **kw):
    if "nc" not in _CACHE:
        _CACHE["nc"] = _build_nc()
    return run_bass_kernel_spmd(
        _CACHE["nc"], in_maps, core_ids=list(range(NCORES)), trace=trace, **kw
    )


def kernel(hidden_states, k_qx, k_scale, v_qx, v_scale, cos, sin, Wq, Wk, Wv, Wo):
    in_maps = _prep_inputs(
        hidden_states, k_qx, k_scale, v_qx, v_scale, cos, sin, Wq, Wk, Wv, Wo
    )
    res = _run(in_maps)
    out = np.zeros((1, 1, HID), np.float32)
    for r in res.results:
        out += r["out"].reshape(1, 1, HID)
    return out


# revision 25
# speedup vs baseline: 1.5438x; 1.1931x over previous
"""Trainium2 Bass kernel for Llama SmartKV decode attention (GQA, q_len=1).

Sharding: tensor-parallel over KV heads — core c owns kv head c and its
GQA group of 4 query heads (slices of Wq/Wk/Wv/Wo), plus that head's
quantized KV cache. Each core computes its partial o_proj output; the
host sums the 8 partials (the all-reduce).

Host-side input prep (pure numpy, done once per call):
  - RoPE (cos/sin) and the 1/sqrt(D) score scale are folded into Wq/Wk.
  - k_scale/v_scale are folded into the int8 code cache, stored fp16.
  - All device operands are fp16 (PE runs fp16 at full rate; fp32 PSUM
    accumulation); softmax exp uses a constant bias which cancels in the
    normalization but keeps exp() in fp16 range for any input scale.
"""

import os

os.environ.setdefault("BY_DEFAULT_DISABLE_SUBTILE_DEPS", "1")

import numpy as np

import concourse.bass as bass
import concourse.mybir as mybir
import concourse.tile as tile
from concourse.bass_utils import run_bass_kernel_spmd

H, HKV, D, HID, S = 32, 8, 128, 4096, 32768
G = H // HKV  # 4 query heads per core
NCORES = 8
KC = HID // 128  # 32 contraction chunks for projections
NCH = S // 128  # 256 score/PV chunks of 128 tokens
KTILE = 8192  # tokens per K-cache DMA tile
VCH = 32  # s-chunks per V-cache DMA tile
WJ = 4  # projection j-chunks per wqkv DMA tile
F16 = mybir.dt.float16
I8 = mybir.dt.int8
F32 = mybir.dt.float32
EXP_BIAS = -11.0  # exp(s + B): cancels in softmax, keeps fp16 in range

_CACHE = {}


def _reduce_dma_waits(nc):
    """Drop transitively-implied waits from DMA instructions.

    The PSEUDO_DMA_DIRECT2D descriptor holds exactly one wait slot, but
    Tile's sem assignment is not transitively minimal (its optimize_sems
    pass is disabled), so pool-slot-recycling DMAs carry a redundant
    second wait: the WAW wait on the previous slot writer is already
    implied by the engine-reader wait.  We verify implication with a
    vector-clock walk over the scheduled program and delete only waits
    that are provably redundant.
    """
    import bass_rust as _br

    insts = []
    for f in nc.m.functions:
        for bb in f.blocks:
            insts.extend(bb.instructions)

    cum = {}  # sem name -> cumulative value so far in schedule order
    snaps = {}  # sem name -> list of (cumval, knowledge dict)
    streams = {}  # stream key -> knowledge dict (sem name -> value known >=)

    def know_at(sem, val):
        # knowledge of the producer that first brought `sem` to >= val
        for cv, kn in snaps.get(sem, ()):
            if cv >= val:
                return kn
        return None

    for inst in insts:
        si = inst.sync_info
        if si is None:
            continue
        waits = list(si.on_wait)
        ups = list(si.on_update)
        if ups and ups[0].ant_name.startswith(("DMASW", "DMAHW")):
            skey = ups[0].ant_name
        else:
            skey = f"eng:{inst.engine}"
        kn = dict(streams.get(skey, ()))

        imm = [
            w
            for w in waits
            if w.wait_mode == "sem-ge-imm" and w.sync_type == "semaphore"
        ]
        if len(imm) == len(waits) > 1:
            keep = []
            for w in waits:
                others = dict(kn)
                for w2 in waits:
                    if w2 is w:
                        continue
                    others[w2.ant_name] = max(
                        others.get(w2.ant_name, 0), w2.wait_value
                    )
                    k2 = know_at(w2.ant_name, w2.wait_value)
                    if k2:
                        for s, v in k2.items():
                            others[s] = max(others.get(s, 0), v)
                if others.get(w.ant_name, 0) >= w.wait_value:
                    continue  # implied: drop
                keep.append(w)
            if len(keep) < len(waits):
                inst.sync_info = _br.SyncInfo(on_wait=keep, on_update=ups)
                waits = keep

        # fold wait knowledge into this instruction's stream knowledge
        for w in waits:
            if w.wait_mode != "sem-ge-imm" or w.sync_type != "semaphore":
                continue
            kn[w.ant_name] = max(kn.get(w.ant_name, 0), w.wait_value)
            k2 = know_at(w.ant_name, w.wait_value)
            if k2:
                for s, v in k2.items():
                    kn[s] = max(kn.get(s, 0), v)
        for u in ups:
            if u.sync_type != "semaphore":
                continue
            cum[u.ant_name] = cum.get(u.ant_name, 0) + u.update_value
            kn[u.ant_name] = max(kn.get(u.ant_name, 0), cum[u.ant_name])
            snaps.setdefault(u.ant_name, []).append((cum[u.ant_name], kn))
        streams[skey] = kn

    bad = [
        (i.name, type(i).__name__, [(w.ant_name, w.wait_value) for w in i.sync_info.on_wait])
        for i in insts
        if i.sync_info is not None
        and len(i.sync_info.on_wait) > 1
        and type(i).__name__ not in ("InstDrain",)
    ]
    if bad:
        raise RuntimeError(f"instructions still multi-wait: {bad}")


def _build_nc(debug=False):
    nc = bass.Bass()
    hsT = nc.declare_dram_parameter("hsT", [128, KC], F16, isOutput=False)
    wqkv = nc.declare_dram_parameter("wqkv", [KC // WJ, 128, WJ * 768], F16, isOutput=False)
    kT = nc.declare_dram_parameter("kT", [128, S], I8, isOutput=False)
    ksc = nc.declare_dram_parameter("ksc", [128, S // 128], F32, isOutput=False)
    vsc = nc.declare_dram_parameter("vsc", [128, S // 128], F32, isOutput=False)
    v8 = nc.declare_dram_parameter("v8", [S // (128 * VCH), 128, VCH * D], I8, isOutput=False)
    wo = nc.declare_dram_parameter("wo", [G, 128, HID], F16, isOutput=False)
    out = nc.declare_dram_parameter("out", [1, HID], F32, isOutput=True)
    if debug:
        dbg = nc.declare_dram_parameter("dbg", [128, 5280], F32, isOutput=True)

    PS = bass.MemorySpace.PSUM
    with tile.TileContext(nc) as tc:
        with (
            tc.tile_pool(name="const", bufs=1) as cpool,
            tc.tile_pool(name="wqkvp", bufs=8) as wqkv_pool,
            tc.tile_pool(name="kp", bufs=3) as k_pool,
            tc.tile_pool(name="vp", bufs=3) as v_pool,
            tc.tile_pool(name="wop", bufs=1) as wo_pool,
            tc.tile_pool(name="sm", bufs=1) as sm,
        ):
            # ---- load hs ----
            hs_sb = cpool.tile([128, KC], F16)
            nc.sync.dma_start(out=hs_sb, in_=hsT[:, :])
            ebias_sb = cpool.tile([128, 1], F32)
            nc.vector.memset(ebias_sb, EXP_BIAS)
            ones_sb = cpool.tile([128, 1], F32)
            nc.vector.memset(ones_sb, 1.0)
            ones_row = cpool.tile([1, 128], F32)
            nc.vector.memset(ones_row, 1.0 / 16384.0)

            qk_sb = sm.tile([128, G + 1], F16)
            vrow_sb = sm.tile([1, D], F32)
            ksc_sb = cpool.tile([128, S // 128], F32)
            nc.gpsimd.dma_start(out=ksc_sb, in_=ksc[:, :])
            vsc_sb = cpool.tile([128, S // 128], F32)
            nc.gpsimd.dma_start(out=vsc_sb, in_=vsc[:, :])
            # tiny DVE reads so later DVE ops inherit the scale-DMA waits
            # through the engine stream (keeps every op single-wait)
            touch_sb = sm.tile([1, 2], F32)
            nc.vector.tensor_copy(out=touch_sb[:, 0:1], in_=ksc_sb[0:1, 0:1])
            nc.vector.tensor_copy(out=touch_sb[:, 1:2], in_=vsc_sb[0:1, 0:1])

            # ---- q/k/v projections (RoPE+scale pre-folded into weights) ----
            # PSUM supports one OPEN accumulation group per bank: each of the
            # 6 concurrently-accumulating outputs (q0..q3, k, v) gets its own
            # bank; the pool scope closes afterwards so later phases reuse them.
            with tc.tile_pool(name="psqk", bufs=1, space=PS) as psqk_pool:
                ps_qk = [
                    psqk_pool.tile([128, 1], F32, name=f"ps_qk{h}", tag=f"qk{h}")
                    for h in range(G + 1)
                ]
                ps_v = psqk_pool.tile([1, D], F32, tag="psv")
                for jj in range(KC // WJ):
                    w_sb = wqkv_pool.tile([128, WJ * 768], F16, tag="wt")
                    nc.gpsimd.dma_start(out=w_sb, in_=wqkv[jj])
                    for c in range(WJ):
                        j = jj * WJ + c
                        for h in range(G + 1):
                            nc.tensor.matmul(
                                ps_qk[h][:, :],
                                lhsT=w_sb[:, c * 768 + h * 128 : c * 768 + (h + 1) * 128],
                                rhs=hs_sb[:, j : j + 1],
                                start=(j == 0),
                                stop=(j == KC - 1),
                            )
                        nc.tensor.matmul(
                            ps_v[:, :],
                            lhsT=hs_sb[:, j : j + 1],
                            rhs=w_sb[:, c * 768 + 640 : c * 768 + 768],
                            start=(j == 0),
                            stop=(j == KC - 1),
                        )
                for h in range(G + 1):
                    nc.vector.tensor_copy(out=qk_sb[:, h : h + 1], in_=ps_qk[h])
                # scaled 2^14 to match the v_scale-folded PV accumulation
                nc.scalar.mul(out=vrow_sb, in_=ps_v, mul=16384.0)

            with (
                tc.tile_pool(name="pssc", bufs=1, space=PS) as pssc_pool,
                tc.tile_pool(name="pspv", bufs=1, space=PS) as pspv_pool,
                tc.tile_pool(name="psms", bufs=1, space=PS) as psms_pool,
            ):
                # ---- current-token score row: s_curT[1, g] = k_cur . q_g ----
                ps_scur = psms_pool.tile([1, G], F32, tag="ms")
                nc.tensor.matmul(
                    ps_scur[:, :], lhsT=qk_sb[:, G : G + 1], rhs=qk_sb[:, 0:G],
                    start=True, stop=True,
                )
                pcur_sb = sm.tile([1, G], F16)
                nc.scalar.activation(
                    out=pcur_sb, in_=ps_scur,
                    func=mybir.ActivationFunctionType.Exp, bias=ebias_sb[:1],
                )
                pcurf_sb = sm.tile([1, G], F32)
                nc.scalar.activation(
                    out=pcurf_sb, in_=ps_scur,
                    func=mybir.ActivationFunctionType.Exp, bias=ebias_sb[:1],
                )

                # ---- scores over the cache: [s, g] layout, 2 PSUM banks ----
                ps_sc = [
                    pssc_pool.tile([128, 512], F32, name=f"ps_sc{b}", tag=f"sc{b}")
                    for b in range(2)
                ]
                probs_sb = [
                    sm.tile([128, 512], F16, name=f"probs{b}", tag=f"pr{b}")
                    for b in range(2)
                ]
                pprime_sb = [
                    sm.tile([128, 512], F16, name=f"pprime{b}", tag=f"pp{b}")
                    for b in range(2)
                ]
                dpart_sb = sm.tile([128, 2 * G], F32)
                kcpt = KTILE // 128  # score chunks per K tile
                for co in range(S // KTILE):
                    k_sb = k_pool.tile([128, KTILE], F16, tag="kt")
                    nc.sync.dma_start(
                        out=k_sb, in_=kT[:, co * KTILE : (co + 1) * KTILE]
                    )
                    for ci in range(kcpt):
                        ch = co * kcpt + ci
                        b, col = ch // 128, (ch % 128) * 4
                        nc.tensor.matmul(
                            ps_sc[b][:, col : col + 4],
                            lhsT=k_sb[:, ci * 128 : (ci + 1) * 128],
                            rhs=qk_sb[:, 0:G],
                            start=True,
                            stop=True,
                        )
                    if (co + 1) * kcpt % 128 == 0:
                        b = ((co + 1) * kcpt - 1) // 128
                        # scores = raw_codes_dot * k_scale[s]  (per-s scale,
                        # broadcast over the 4 g columns)
                        kb = ksc_sb[:, b * 128 : (b + 1) * 128]
                        kb_bc = bass.AP(tensor=kb.tensor, offset=kb.offset,
                                        ap=[*kb.ap, [0, G]])
                        scraw = sm.tile([128, 512], F32, name=f"scraw{b}", tag="scr", bufs=2)
                        nc.vector.tensor_mul(
                            out=scraw.rearrange("p (c g) -> p c g", g=G),
                            in0=ps_sc[b].rearrange("p (c g) -> p c g", g=G),
                            in1=kb_bc,
                        )
                        nc.scalar.activation(
                            out=probs_sb[b], in_=scraw,
                            func=mybir.ActivationFunctionType.Exp, bias=ebias_sb,
                        )
                        # per-(partition, g) partials: reduce over the 128
                        # chunk-columns (stride 4) of the bank
                        nc.vector.reduce_sum(
                            out=dpart_sb[:, b * G : (b + 1) * G],
                            in_=probs_sb[b].rearrange("p (c g) -> p g c", g=G),
                            axis=mybir.AxisListType.X,
                        )
                        # fold v_scale[s] into the probabilities used by PV
                        vb = vsc_sb[:, b * 128 : (b + 1) * 128]
                        vb_bc = bass.AP(tensor=vb.tensor, offset=vb.offset,
                                        ap=[*vb.ap, [0, G]])
                        nc.vector.tensor_mul(
                            out=pprime_sb[b].rearrange("p (c g) -> p c g", g=G),
                            in0=probs_sb[b].rearrange("p (c g) -> p c g", g=G),
                            in1=vb_bc,
                        )

                # o_proj weights: issued on sync AFTER the kT triggers so
                # they arrive once the score stream has drained
                wo_sb = [
                    wo_pool.tile([128, HID], F16, name=f"wo_sb{g}", tag=f"wo{g}")
                    for g in range(G)
                ]
                for g in range(G):
                    nc.sync.dma_start(out=wo_sb[g], in_=wo[g])

                # ---- PV: outT[d, g] over all 256 chunks + current token ----
                ps_pv = pspv_pool.tile([128, G], F32)
                for co in range(S // (128 * VCH)):
                    v_sb = v_pool.tile([128, VCH * D], F16, tag="vt")
                    nc.scalar.dma_start(out=v_sb, in_=v8[co])
                    for ci in range(VCH):
                        ch = co * VCH + ci
                        b, col = ch // 128, (ch % 128) * 4
                        nc.tensor.matmul(
                            ps_pv[:, :],
                            lhsT=v_sb[:, ci * D : (ci + 1) * D],
                            rhs=pprime_sb[b][:, col : col + 4],
                            start=(ch == 0),
                            stop=False,
                        )
                nc.tensor.matmul(
                    ps_pv[:, :], lhsT=vrow_sb, rhs=pcurf_sb, start=False, stop=True,
                )

                # ---- denominator and normalization ----
                ps_den = psms_pool.tile([1, 2 * G], F32, tag="ms")
                nc.tensor.matmul(
                    ps_den[:, :], lhsT=ones_sb, rhs=dpart_sb, start=True, stop=True,
                )
                den_sb = sm.tile([1, 2 * G], F32)
                nc.vector.tensor_copy(out=den_sb, in_=ps_den)
                dtot_sb = sm.tile([1, G], F32)
                nc.vector.tensor_add(
                    out=dtot_sb, in0=den_sb[:, 0:G], in1=den_sb[:, G : 2 * G]
                )
                nc.vector.tensor_add(out=dtot_sb, in0=dtot_sb, in1=pcurf_sb)
                rden_sb = sm.tile([1, G], F32)
                nc.vector.reciprocal(out=rden_sb, in_=dtot_sb)
                # broadcast rden across partitions on PE (ones outer product)
                # so the normalize's waits all chain through one engine
                ps_bc = psms_pool.tile([128, G], F32, tag="ms")
                nc.tensor.matmul(
                    ps_bc[:, :], lhsT=ones_row, rhs=rden_sb, start=True, stop=True
                )
                bc_sb = sm.tile([128, G], F32)
                nc.vector.tensor_copy(out=bc_sb, in_=ps_bc)
                outn_sb = sm.tile([128, G], F16)
                nc.vector.tensor_mul(out=outn_sb, in0=ps_pv, in1=bc_sb)
                if debug:
                    pv_sb = sm.tile([128, G], F32)
                    nc.vector.tensor_copy(out=pv_sb, in_=ps_pv)

            # ---- o_proj: o[1, HID] = sum_g outT[:, g]^T @ Wo[g] ----
            with tc.tile_pool(name="pso", bufs=2, space=PS) as pso_pool:
                ofin_sb = sm.tile([1, HID], F32)
                for n in range(HID // 512):
                    ps_on = pso_pool.tile([1, 512], F32, tag="on")
                    for g in range(G):
                        nc.tensor.matmul(
                            ps_on[:, :],
                            lhsT=outn_sb[:, g : g + 1],
                            rhs=wo_sb[g][:, n * 512 : (n + 1) * 512],
                            start=(g == 0),
                            stop=(g == G - 1),
                        )
                    nc.scalar.copy(out=ofin_sb[:, n * 512 : (n + 1) * 512], in_=ps_on)
            if not debug:
                nc.gpsimd.dma_start(out=out[:, :], in_=ofin_sb)
            else:
                mega = sm.tile([128, 5280], F32)
                nc.vector.tensor_copy(out=mega[:, 0:512], in_=probs_sb[0])
                nc.vector.tensor_copy(out=mega[:, 512:1024], in_=probs_sb[1])
                nc.vector.tensor_copy(out=mega[:, 1024:1029], in_=qk_sb)
                nc.vector.tensor_copy(out=mega[:, 1029:1037], in_=dpart_sb)
                nc.vector.tensor_copy(out=mega[:, 1037:1041], in_=pv_sb)
                nc.vector.tensor_copy(out=mega[:, 1041:1045], in_=outn_sb)
                nc.vector.tensor_copy(out=mega[0:1, 1045:1173], in_=vrow_sb)
                nc.vector.tensor_copy(out=mega[0:1, 1173:1177], in_=dtot_sb)
                nc.vector.tensor_copy(out=mega[0:1, 1177:1181], in_=pcur_sb)
                nc.vector.tensor_copy(out=mega[0:1, 1184:5280], in_=ofin_sb)
                nc.gpsimd.dma_start(out=dbg[:, :], in_=mega)

    _reduce_dma_waits(nc)
    return nc


def _rope_fold(W, nheads, cos, sin, scale=1.0):
    """Fold RoPE rotation (and an optional scalar) into projection weights."""
    W = W.reshape(HID, nheads, D).astype(np.float32)
    half = D // 2
    Wr = np.empty_like(W)
    Wr[:, :, :half] = cos[:half] * W[:, :, :half] - sin[:half] * W[:, :, half:]
    Wr[:, :, half:] = cos[half:] * W[:, :, half:] + sin[half:] * W[:, :, :half]
    return (Wr * np.float32(scale)).reshape(HID, nheads * D)


def _prep_inputs(hidden_states, k_qx, k_scale, v_qx, v_scale, cos, sin, Wq, Wk, Wv, Wo):
    f16 = np.float16
    hs = np.ascontiguousarray(hidden_states.reshape(HID)).astype(np.float32)
    cos = cos.astype(np.float32)
    sin = sin.astype(np.float32)
    Wq_f = _rope_fold(Wq, H, cos, sin, 1.0 / np.sqrt(D))
    Wk_f = _rope_fold(Wk, HKV, cos, sin)
    hsT = np.ascontiguousarray(hs.reshape(KC, 128).T).astype(f16)

    in_maps = []
    for c in range(NCORES):
        qcols = slice(G * c * D, G * (c + 1) * D)
        kvcols = slice(c * D, (c + 1) * D)
        wqkv = np.ascontiguousarray(
            np.concatenate([Wq_f[:, qcols], Wk_f[:, kvcols], Wv[:, kvcols]], axis=1)
            .astype(f16)
            .reshape(KC // 4, 4, 128, 768)
            .transpose(0, 2, 1, 3)
        ).reshape(KC // 4, 128, 4 * 768)
        kT = np.ascontiguousarray(k_qx[:, c, :].astype(np.int8).T)
        v8 = np.ascontiguousarray(
            v_qx[:, c, :].astype(np.int8)
            .reshape(S // (128 * VCH), VCH, 128, D)
            .transpose(0, 2, 1, 3)
        ).reshape(S // (128 * VCH), 128, VCH * D)
        ksc = np.ascontiguousarray(k_scale[:, c, 0].astype(np.float32).reshape(S // 128, 128).T)
        vsc = np.ascontiguousarray(v_scale[:, c, 0].astype(np.float32).reshape(S // 128, 128).T) * np.float32(16384.0)
        wo = np.ascontiguousarray(
            Wo[G * c * D : G * (c + 1) * D, :].astype(f16)
        ).reshape(G, 128, HID)
        in_maps.append(
            {"hsT": hsT, "wqkv": wqkv, "kT": kT, "v8": v8, "wo": wo,
             "ksc": ksc, "vsc": vsc}
        )
    return in_maps


def _run(in_maps, trace=False, **kw):
    if "nc" not in _CACHE:
        _CACHE["nc"] = _build_nc()
    return run_bass_kernel_spmd(
        _CACHE["nc"], in_maps, core_ids=list(range(NCORES)), trace=trace, **kw
    )


def kernel(hidden_states, k_qx, k_scale, v_qx, v_scale, cos, sin, Wq, Wk, Wv, Wo):
    in_maps = _prep_inputs(
        hidden_states, k_qx, k_scale, v_qx, v_scale, cos, sin, Wq, Wk, Wv, Wo
    )
    res = _run(in_maps)
    out = np.zeros((1, 1, HID), np.float32)
    for r in res.results:
        out += r["out"].reshape(1, 1, HID)
    return out
